# revision 7
# baseline (speedup 1.0000x reference)
"""Trainium2 Bass kernel for nn_ExtractModel (conv-context log-softmax costs).

Math restructuring: the per-(v,l) conv + einsum collapses into gathers from
six precomputed [K, LO] tables P_kw^b = E @ (conv_w[:,:,kw]^T-contracted lost),
plus the unit log-prob table U.  Per output row (v,l):

    A_true[u]  = sum_kw P_kw[idx[v, l+kw-1], u] * valid(l+kw-1)
    out        = -(0.2*(A_true - lse(A_true)) + 0.8*U[idx[v,l]])

Tables are stored pre-scaled and row-max-subtracted:
    T_kw = -0.2*(P_kw - rowmax(P_kw))      (>= 0, fp16)
so gathered sums give A_G = -0.2*(A_true - B) with B = sum of gathered
rowmaxes.  exp(A_true - B) = exp(-5*A_G) needs no per-row max pass, and B
cancels in the final value:  out = A_G + 0.2*log(sum exp) + (-0.8*U[idx]).

Device work per core (500 vocab rows = 6000 output rows, padded to 6144):
  - PE preamble (bf16): lost, unit logits L1, U log-softmax, 6 P tables.
  - 3 dma_gather streams per 512-row supertile: T0-pair / T2-pair (rows
    [T_sub | T_ins], zero row for masked), T1U (rows [T1_sub | T1_ins | -0.8U],
    double-height so masked l still fetches its U chunk).
  - DVE: two fp16 adds (both branches concatenated), final scalar_tensor_tensor.
  - ACT: exp with fused row-sum accumulation, log.
Outputs sub/ins written fp16 and upcast on host; alignment = exp(U) in f32.
"""

import os
import sys
from contextlib import ExitStack

import numpy as np

for _p in ("/opt/trn_rl_repo", "/root/.axon_site/_ro/trn_rl_repo"):
    if os.path.isdir(_p) and _p not in sys.path:
        sys.path.insert(0, _p)

import ml_dtypes  # noqa: E402

NCORES = 8
V, L, K, LO, D = 4000, 12, 1024, 1024, 512
VPC = V // NCORES            # 500 vocab words per core
ROWS = VPC * L               # 6000 output rows per core
NI = 512                     # rows per gather / supertile
NST = 12                     # supertiles per core
PROWS = NI * NST             # 6144 (padded rows)
CT = NI // 128               # 4 chunks per supertile
MW = NI // 16                # 32 idx columns per supertile
CW = 0.2                     # context weight

_CACHE = {}


def _build_nc():
    import concourse.bacc as bacc
    import concourse.tile as tile
    from concourse import mybir

    f16 = mybir.dt.float16
    bf16 = mybir.dt.bfloat16
    f32 = mybir.dt.float32
    i16 = mybir.dt.int16
    AF = mybir.ActivationFunctionType
    OP = mybir.AluOpType
    X = mybir.AxisListType.X

    nc = bacc.Bacc("TRN2", target_bir_lowering=False, debug=False,
                   num_devices=NCORES)

    E_in = nc.dram_tensor("E", [K, D], f32, kind="ExternalInput")
    ET_in = nc.dram_tensor("ET", [D, K], f32, kind="ExternalInput")
    ALT_in = nc.dram_tensor("ALT", [K, LO], f32, kind="ExternalInput")
    W_in = {}
    for b in ("s", "i"):
        for k in range(3):
            W_in[(b, k)] = nc.dram_tensor(f"W{b}{k}", [D, D], bf16,
                                          kind="ExternalInput")
    idx_in = {}
    for nm in ("idx0", "idx1", "idx2"):
        idx_in[nm] = nc.dram_tensor(nm, [128, MW * NST], i16,
                                    kind="ExternalInput")

    sub_out = nc.dram_tensor("sub", [PROWS, LO], f16, kind="ExternalOutput")
    ins_out = nc.dram_tensor("ins", [PROWS, LO], f16, kind="ExternalOutput")
    ali_out = nc.dram_tensor("ali", [K, LO], f32, kind="ExternalOutput")

    T0P = nc.dram_tensor("T0P", [K + 1, 2 * LO], f16, kind="Internal")
    T2P = nc.dram_tensor("T2P", [K + 1, 2 * LO], f16, kind="Internal")
    T1U = nc.dram_tensor("T1U", [2 * K, 3 * LO], f16, kind="Internal")

    with tile.TileContext(nc) as tc, ExitStack() as ctx:
        const = ctx.enter_context(tc.tile_pool(name="const", bufs=1))
        idx_sb = {}
        for nm in ("idx0", "idx1", "idx2"):
            t = const.tile([128, MW * NST], i16, tag=nm)
            nc.sync.dma_start(t[:], idx_in[nm][:])
            idx_sb[nm] = t

        # ============ PREAMBLE (scoped pools, released before main) ========
        with tc.tile_pool(name="pre", bufs=1) as pre, \
             tc.tile_pool(name="pstg", bufs=2) as pstg, \
             tc.tile_pool(name="ptiny", bufs=4) as ptiny, \
             tc.tile_pool(name="psc", bufs=2) as psc, \
             tc.tile_pool(name="pp", bufs=2, space="PSUM") as pp:

            E32 = pre.tile([128, 8, D], f32)
            nc.sync.dma_start(
                E32[:], E_in[:].rearrange("(kc p) d -> p kc d", p=128))
            ET32 = pre.tile([128, 4, K], f32)
            nc.sync.dma_start(
                ET32[:], ET_in[:].rearrange("(kc p) d -> p kc d", p=128))
            ALT32 = pre.tile([128, 8, LO], f32)
            nc.sync.dma_start(
                ALT32[:], ALT_in[:].rearrange("(kc p) d -> p kc d", p=128))
            zero2k = pre.tile([128, 2 * LO], f16)
            nc.vector.memset(zero2k[:], 0.0)
            lostT32 = pre.tile([128, 4, LO], f32)
            lostT_sb = pre.tile([128, 4, LO], bf16)
            ET_sb = pre.tile([128, 4, K], bf16)
            for Mb in range(4):
                nc.scalar.activation(ET_sb[:, Mb, :], ET32[:, Mb, :],
                                     AF.Identity)

            def mm_acc(ps_ap, lhsT_sl, rhs_sl):
                n = len(lhsT_sl)
                for kc in range(n):
                    nc.tensor.matmul(ps_ap, lhsT_sl[kc], rhs_sl[kc],
                                     start=(kc == 0), stop=(kc == n - 1))

            # ---- lost_T[d, u] = sum_K E[Kk,d] * alignerT[Kk,u]  (true f32) ----
            for Mb in range(4):
                for Nb in range(2):
                    ps = pp.tile([128, 512], f32, space="PSUM", tag="psA")
                    mm_acc(ps[:],
                           [E32[:, kc, Mb * 128:(Mb + 1) * 128]
                            for kc in range(8)],
                           [ALT32[:, kc, Nb * 512:(Nb + 1) * 512]
                            for kc in range(8)])
                    nc.scalar.activation(
                        lostT32[:, Mb, Nb * 512:(Nb + 1) * 512], ps[:],
                        AF.Identity)
                nc.scalar.activation(lostT_sb[:, Mb, :], lostT32[:, Mb, :],
                                     AF.Identity)

            # ---- L1 = E @ lost_T rows (true f32); U softmax; TU/alignment ----
            for Mb in range(8):
                psA = pp.tile([128, 512], f32, space="PSUM", tag="psA")
                psB = pp.tile([128, 512], f32, space="PSUM", tag="psB")
                lh = [ET32[:, kc, Mb * 128:(Mb + 1) * 128] for kc in range(4)]
                mm_acc(psA[:], lh, [lostT32[:, kc, 0:512] for kc in range(4)])
                mm_acc(psB[:], lh,
                       [lostT32[:, kc, 512:1024] for kc in range(4)])
                m2 = ptiny.tile([128, 2], f32, tag="m2")
                nc.vector.tensor_reduce(m2[:, 0:1], psA[:], axis=X, op=OP.max)
                nc.vector.tensor_reduce(m2[:, 1:2], psB[:], axis=X, op=OP.max)
                mx = ptiny.tile([128, 1], f32, tag="mx")
                nc.vector.tensor_tensor(mx[:], m2[:, 0:1], m2[:, 1:2],
                                        op=OP.max)
                nmx = ptiny.tile([128, 1], f32, tag="nmx")
                nc.vector.tensor_scalar_mul(nmx[:], mx[:], -1.0)
                s2 = ptiny.tile([128, 2], f32, tag="s2")
                ex = psc.tile([128, 1024], f32, tag="ex")
                nc.scalar.activation(ex[:, 0:512], psA[:], AF.Exp,
                                     bias=nmx[:, 0:1], accum_out=s2[:, 0:1])
                nc.scalar.activation(ex[:, 512:1024], psB[:], AF.Exp,
                                     bias=nmx[:, 0:1], accum_out=s2[:, 1:2])
                s = ptiny.tile([128, 1], f32, tag="s")
                nc.vector.tensor_tensor(s[:], s2[:, 0:1], s2[:, 1:2], op=OP.add)
                rs = ptiny.tile([128, 1], f32, tag="rs")
                nc.vector.reciprocal(rs[:], s[:])
                alw = psc.tile([128, 1024], f32, tag="alw")
                nc.vector.tensor_scalar(alw[:], ex[:], scalar1=rs[:, 0:1],
                                        scalar2=None, op0=OP.mult)
                nc.sync.dma_start(ali_out[Mb * 128:(Mb + 1) * 128, :], alw[:])
                ls = ptiny.tile([128, 1], f32, tag="ls")
                nc.scalar.activation(ls[:], s[:], AF.Ln)
                bU = ptiny.tile([128, 1], f32, tag="bU")
                nc.vector.tensor_tensor(bU[:], mx[:], ls[:], op=OP.add)
                bU8 = ptiny.tile([128, 1], f32, tag="bU8")
                nc.vector.tensor_scalar_mul(bU8[:], bU[:], 0.8)
                tu = pstg.tile([128, LO], f16, tag="tu")
                nc.scalar.activation(tu[:, 0:512], psA[:], AF.Identity,
                                     bias=bU8[:, 0:1], scale=-0.8)
                nc.scalar.activation(tu[:, 512:1024], psB[:], AF.Identity,
                                     bias=bU8[:, 0:1], scale=-0.8)
                r0, r1 = Mb * 128, (Mb + 1) * 128
                nc.sync.dma_start(T1U[r0:r1, 2 * LO:3 * LO], tu[:])
                nc.sync.dma_start(T1U[K + r0:K + r1, 2 * LO:3 * LO], tu[:])
                nc.sync.dma_start(T1U[K + r0:K + r1, 0:2 * LO], zero2k[:])

            # zero rows of the pair tables
            nc.sync.dma_start(T0P[K:K + 1, :], zero2k[0:1, :])
            nc.sync.dma_start(T2P[K:K + 1, :], zero2k[0:1, :])

            # ---- M_kw^b and P tables ----
            for k in range(3):
                Ms = {}
                for b in ("s", "i"):
                    Wt = pstg.tile([128, 4, D], bf16, tag="Wt")
                    nc.sync.dma_start(
                        Wt[:],
                        W_in[(b, k)][:].rearrange("(kc p) d -> p kc d", p=128))
                    M = pstg.tile([128, 4, LO], bf16, tag=f"M{b}")
                    for Mb in range(4):
                        for Nb in range(2):
                            ps = pp.tile([128, 512], f32, space="PSUM",
                                         tag="psA")
                            mm_acc(ps[:],
                                   [Wt[:, kc, Mb * 128:(Mb + 1) * 128]
                                    for kc in range(4)],
                                   [lostT_sb[:, kc, Nb * 512:(Nb + 1) * 512]
                                    for kc in range(4)])
                            nc.scalar.activation(
                                M[:, Mb, Nb * 512:(Nb + 1) * 512], ps[:],
                                AF.Identity)
                    Ms[b] = M
                for Mb in range(8):
                    stage = pstg.tile([128, 2 * LO], f16, tag="stage")
                    lh = [ET_sb[:, kc, Mb * 128:(Mb + 1) * 128]
                          for kc in range(4)]
                    for bi, b in enumerate(("s", "i")):
                        psA = pp.tile([128, 512], f32, space="PSUM", tag="psA")
                        psB = pp.tile([128, 512], f32, space="PSUM", tag="psB")
                        mm_acc(psA[:], lh,
                               [Ms[b][:, kc, 0:512] for kc in range(4)])
                        mm_acc(psB[:], lh,
                               [Ms[b][:, kc, 512:1024] for kc in range(4)])
                        m2 = ptiny.tile([128, 2], f32, tag="m2")
                        nc.vector.tensor_reduce(m2[:, 0:1], psA[:], axis=X,
                                                op=OP.max)
                        nc.vector.tensor_reduce(m2[:, 1:2], psB[:], axis=X,
                                                op=OP.max)
                        mx = ptiny.tile([128, 1], f32, tag="mx")
                        nc.vector.tensor_tensor(mx[:], m2[:, 0:1], m2[:, 1:2],
                                                op=OP.max)
                        b02 = ptiny.tile([128, 1], f32, tag="b02")
                        nc.vector.tensor_scalar_mul(b02[:], mx[:], CW)
                        co = bi * LO
                        nc.scalar.activation(stage[:, co:co + 512], psA[:],
                                             AF.Identity, bias=b02[:, 0:1],
                                             scale=-CW)
                        nc.scalar.activation(stage[:, co + 512:co + 1024],
                                             psB[:], AF.Identity,
                                             bias=b02[:, 0:1], scale=-CW)
                    r0, r1 = Mb * 128, (Mb + 1) * 128
                    if k == 0:
                        nc.sync.dma_start(T0P[r0:r1, :], stage[:])
                    elif k == 2:
                        nc.sync.dma_start(T2P[r0:r1, :], stage[:])
                    else:
                        nc.sync.dma_start(T1U[r0:r1, 0:2 * LO], stage[:])

        # ===================== MAIN LOOP =====================
        with tc.tile_pool(name="gath", bufs=2) as gp, \
             tc.tile_pool(name="outp", bufs=2) as op_, \
             tc.tile_pool(name="msc", bufs=2) as msc, \
             tc.tile_pool(name="mtiny", bufs=4) as mtiny:
            for st in range(NST):
                sl = slice(st * MW, (st + 1) * MW)
                G1 = gp.tile([128, CT, 3 * LO], f16, tag="G1")
                nc.gpsimd.dma_gather(out_ap=G1[:], in_ap=T1U[:],
                                     idxs_ap=idx_sb["idx1"][:, sl],
                                     num_idxs=NI, num_idxs_reg=NI,
                                     elem_size=3 * LO)
                G0 = gp.tile([128, CT, 2 * LO], f16, tag="G0")
                nc.gpsimd.dma_gather(out_ap=G0[:], in_ap=T0P[:],
                                     idxs_ap=idx_sb["idx0"][:, sl],
                                     num_idxs=NI, num_idxs_reg=NI,
                                     elem_size=2 * LO)
                G2 = gp.tile([128, CT, 2 * LO], f16, tag="G2")
                nc.gpsimd.dma_gather(out_ap=G2[:], in_ap=T2P[:],
                                     idxs_ap=idx_sb["idx2"][:, sl],
                                     num_idxs=NI, num_idxs_reg=NI,
                                     elem_size=2 * LO)
                nc.vector.tensor_tensor(G0[:], G0[:], G1[:, :, 0:2 * LO],
                                        op=OP.add)
                nc.vector.tensor_tensor(G0[:], G0[:], G2[:], op=OP.add)
                # per-row min of A_G == -CW*(max(A_true) - B): exact exp guard
                mn8 = mtiny.tile([128, 2 * CT], f32, tag="mn8")
                for c in range(CT):
                    for bi in range(2):
                        j = 2 * c + bi
                        nc.vector.tensor_reduce(
                            mn8[:, j:j + 1], G0[:, c, bi * LO:(bi + 1) * LO],
                            axis=X, op=OP.min)
                bn8 = mtiny.tile([128, 2 * CT], f32, tag="bn8")
                nc.vector.tensor_scalar_mul(bn8[:], mn8[:], 1.0 / CW)
                s8 = mtiny.tile([128, 2 * CT], f32, tag="s8")
                for c in range(CT):
                    for bi in range(2):
                        j = 2 * c + bi
                        eo = msc.tile([128, LO], f16, tag="eo")
                        nc.scalar.activation(
                            eo[:], G0[:, c, bi * LO:(bi + 1) * LO], AF.Exp,
                            scale=-1.0 / CW, bias=bn8[:, j:j + 1],
                            accum_out=s8[:, j:j + 1])
                cls = mtiny.tile([128, 2 * CT], f32, tag="cls")
                nc.scalar.activation(cls[:], s8[:], AF.Ln)
                cls2 = mtiny.tile([128, 2 * CT], f32, tag="cls2")
                nc.vector.tensor_scalar_mul(cls2[:], cls[:], CW)
                nc.vector.tensor_tensor(cls2[:], cls2[:], mn8[:],
                                        op=OP.subtract)
                Os = op_.tile([128, CT, LO], f16, tag="Os")
                Oi = op_.tile([128, CT, LO], f16, tag="Oi")
                for c in range(CT):
                    nc.vector.scalar_tensor_tensor(
                        Os[:, c, :], G0[:, c, 0:LO],
                        cls2[:, 2 * c:2 * c + 1], G1[:, c, 2 * LO:3 * LO],
                        op0=OP.add, op1=OP.add)
                    nc.vector.scalar_tensor_tensor(
                        Oi[:, c, :], G0[:, c, LO:2 * LO],
                        cls2[:, 2 * c + 1:2 * c + 2], G1[:, c, 2 * LO:3 * LO],
                        op0=OP.add, op1=OP.add)
                sv = sub_out[st * NI:(st + 1) * NI, :].rearrange(
                    "(c p) n -> p c n", p=128)
                iv = ins_out[st * NI:(st + 1) * NI, :].rearrange(
                    "(c p) n -> p c n", p=128)
                nc.sync.dma_start(sv, Os[:])
                nc.sync.dma_start(iv, Oi[:])

    nc.compile()
    return nc


def _get_nc():
    if "nc" not in _CACHE:
        _CACHE["nc"] = _build_nc()
    return _CACHE["nc"]


def _wrap_idx(a):
    """Flat [PROWS] int array -> [128, MW*NST] int16 gather-index layout."""
    out = np.empty((128, MW * NST), np.int16)
    for st in range(NST):
        w = a[st * NI:(st + 1) * NI].reshape(MW, 16).T.astype(np.int16)
        out[:, st * MW:(st + 1) * MW] = np.tile(w, (8, 1))
    return out


def host_prep(known_unit_emb, unit_aligner_weight, conv_w, conv_b,
              ins_conv_w, ins_conv_b, indexed_segments, vocab_length):
    """Build the per-core input maps (numpy only)."""
    E = np.asarray(known_unit_emb, np.float32)
    AL = np.asarray(unit_aligner_weight, np.float32)
    Wc = np.asarray(conv_w, np.float32)
    Wi = np.asarray(ins_conv_w, np.float32)
    idx = np.asarray(indexed_segments).astype(np.int64)
    ln = np.asarray(vocab_length).astype(np.int64)

    bf = ml_dtypes.bfloat16
    E_b = np.ascontiguousarray(E)
    ET_b = np.ascontiguousarray(E.T)
    ALT_b = np.ascontiguousarray(AL.T)
    W_b = {}
    for b, W in (("s", Wc), ("i", Wi)):
        for k in range(3):
            W_b[(b, k)] = np.ascontiguousarray(W[:, :, k], bf)

    lpos = np.arange(L)
    valid = lpos[None, :] < ln[:, None]                      # [V, L]
    prev = np.full((V, L), K, np.int64)
    prev[:, 1:] = np.where(valid[:, :-1], idx[:, :-1], K)
    nxt = np.full((V, L), K, np.int64)
    nxt[:, :-1] = np.where(valid[:, 1:], idx[:, 1:], K)
    cur = np.where(valid, idx, K + idx)                      # T1U double-height

    in_maps = []
    for c in range(NCORES):
        vs = slice(c * VPC, (c + 1) * VPC)
        m = {"E": E_b, "ET": ET_b, "ALT": ALT_b}
        for (b, k), w in W_b.items():
            m[f"W{b}{k}"] = w
        for nm, arr, pad in (("idx0", prev, K), ("idx1", cur, K),
                             ("idx2", nxt, K)):
            flat = np.full(PROWS, pad, np.int64)
            flat[:ROWS] = arr[vs].reshape(-1)
            m[nm] = _wrap_idx(flat)
        in_maps.append(m)
    return in_maps


def assemble(results):
    sub = np.concatenate(
        [r["sub"][:ROWS].astype(np.float32).reshape(VPC, L, LO)
         for r in results], axis=0)
    ins = np.concatenate(
        [r["ins"][:ROWS].astype(np.float32).reshape(VPC, L, LO)
         for r in results], axis=0)
    ali = results[0]["ali"].astype(np.float32)
    return sub, ins, ali


def kernel(**inputs):
    from concourse.bass_utils import run_bass_kernel_spmd
    nc = _get_nc()
    in_maps = host_prep(**inputs)
    res = run_bass_kernel_spmd(nc, in_maps, core_ids=list(range(NCORES)))
    return assemble(res.results)


# revision 14
# speedup vs baseline: 1.1364x; 1.1364x over previous
"""Trainium2 Bass kernel for nn_ExtractModel (conv-context log-softmax costs).

Math restructuring: the per-(v,l) conv + einsum collapses into gathers from
six precomputed [K, LO] tables P_kw^b = E @ (conv_w[:,:,kw]-contracted lost),
plus the unit log-prob table U.  Per output row (v,l):

    A_true[u]  = sum_kw P_kw[idx[v, l+kw-1], u] * valid(l+kw-1)
    out        = -(0.2*(A_true - lse(A_true)) + 0.8*U[idx[v,l]])

Tables are stored pre-scaled and row-max-subtracted:
    T_kw = -0.2*(P_kw - rowmax(P_kw))      (>= 0, fp16)
so the gathered sum gives A_G = -0.2*(A_true - B) with B = the sum of the
gathered rowmaxes, and B cancels in the final value:

    out = A_G + 0.2*log(sum_u exp((A_true - maxA))) - min(A_G) + (-0.8*U[idx])

(min(A_G) = -0.2*(maxA - B) supplies the exact exp guard; computed from a
stride-4 subsample, which bounds the exp argument well within fp32 range.)

Device work per core (500 vocab words = 6000 output rows, padded to 6144):
  - PE: bf16 lost/L1/P-table chain first (tables ready early), then an
    exact-f32 lost + per-core 128-row shard of L1 for the alignment output.
  - 3 dma_gather streams per 512-row supertile: T1U (rows
    [T1_sub | T1_ins | -0.8*U], double-height so masked l still fetches its
    U chunk), T0-pair and T2-pair (rows [T_sub | T_ins], zero row masking).
  - DVE: two fp16 adds (branches concatenated), stride-4 min, final adds.
  - ACT: exp with fused row-sum accumulation, log.
Outputs sub/ins written fp16 and upcast on host; alignment sharded f32.
"""

import os
import sys
from contextlib import ExitStack

import numpy as np

for _p in ("/opt/trn_rl_repo", "/root/.axon_site/_ro/trn_rl_repo"):
    if os.path.isdir(_p) and _p not in sys.path:
        sys.path.insert(0, _p)

import ml_dtypes  # noqa: E402

NCORES = 8
V, L, K, LO, D = 4000, 12, 1024, 1024, 512
VPC = V // NCORES            # 500 vocab words per core
ROWS = VPC * L               # 6000 output rows per core
NI = 512                     # rows per gather / supertile
NST = 12                     # supertiles per core
PROWS = NI * NST             # 6144 (padded rows)
CT = NI // 128               # 4 chunks per supertile
MW = NI // 16                # 32 idx columns per supertile
CW = 0.2                     # context weight

_CACHE = {}


def _build_nc():
    import concourse.bass as bass
    import concourse.bacc as bacc
    import concourse.tile as tile
    from concourse import mybir

    f16 = mybir.dt.float16
    bf16 = mybir.dt.bfloat16
    f32 = mybir.dt.float32
    i16 = mybir.dt.int16
    AF = mybir.ActivationFunctionType
    OP = mybir.AluOpType
    X = mybir.AxisListType.X

    nc = bacc.Bacc("TRN2", target_bir_lowering=False, debug=False,
                   num_devices=NCORES)

    # bf16 operands for the table chain
    Eb_in = nc.dram_tensor("Eb", [K, D], bf16, kind="ExternalInput")
    ETb_in = nc.dram_tensor("ETb", [D, K], bf16, kind="ExternalInput")
    ALTb_in = nc.dram_tensor("ALTb", [K, LO], bf16, kind="ExternalInput")
    # f32 operands for the exact alignment path
    E_in = nc.dram_tensor("E", [K, D], f32, kind="ExternalInput")
    ALT_in = nc.dram_tensor("ALT", [K, LO], f32, kind="ExternalInput")
    ETsh_in = nc.dram_tensor("ETsh", [D, 128], f32, kind="ExternalInput")
    W_in = {}
    for b in ("s", "i"):
        for k in range(3):
            W_in[(b, k)] = nc.dram_tensor(f"W{b}{k}", [D, D], bf16,
                                          kind="ExternalInput")
    idx_in = {}
    for nm in ("idx0", "idx1", "idx2"):
        idx_in[nm] = nc.dram_tensor(nm, [128, MW * NST], i16,
                                    kind="ExternalInput")

    sub_out = nc.dram_tensor("sub", [PROWS, LO], f16, kind="ExternalOutput")
    ins_out = nc.dram_tensor("ins", [PROWS, LO], f16, kind="ExternalOutput")
    ali_out = nc.dram_tensor("ali", [128, LO], f32, kind="ExternalOutput")

    T0P = nc.dram_tensor("T0P", [K + 1, 2 * LO], f16, kind="Internal")
    T2P = nc.dram_tensor("T2P", [K + 1, 2 * LO], f16, kind="Internal")
    T1U = nc.dram_tensor("T1U", [2 * K, 3 * LO], f16, kind="Internal")

    with tile.TileContext(nc) as tc, ExitStack() as ctx:
        const = ctx.enter_context(tc.tile_pool(name="const", bufs=1))
        idx_sb = {}
        for nm in ("idx0", "idx1", "idx2"):
            t = const.tile([128, MW * NST], i16, tag=nm)
            nc.sync.dma_start(t[:], idx_in[nm][:])
            idx_sb[nm] = t

        # ============ PREAMBLE (scoped pools, released before main) ========
        with tc.tile_pool(name="pre", bufs=1) as pre, \
             tc.tile_pool(name="pstg", bufs=2) as pstg, \
             tc.tile_pool(name="ptiny", bufs=4) as ptiny, \
             tc.tile_pool(name="pp", bufs=2, space="PSUM") as pp:

            Eb = pre.tile([128, 8, D], bf16)
            nc.sync.dma_start(
                Eb[:], Eb_in[:].rearrange("(kc p) d -> p kc d", p=128))
            ETb = pre.tile([128, 4, K], bf16)
            nc.sync.dma_start(
                ETb[:], ETb_in[:].rearrange("(kc p) d -> p kc d", p=128))
            ALTb = pre.tile([128, 8, LO], bf16)
            nc.sync.dma_start(
                ALTb[:], ALTb_in[:].rearrange("(kc p) d -> p kc d", p=128))
            zero2k = pre.tile([128, 2 * LO], f16)
            nc.vector.memset(zero2k[:], 0.0)
            lostT_b = pre.tile([128, 4, LO], bf16)

            def mm_acc(ps_ap, lhsT_sl, rhs_sl):
                n = len(lhsT_sl)
                for kc in range(n):
                    nc.tensor.matmul(ps_ap, lhsT_sl[kc], rhs_sl[kc],
                                     start=(kc == 0), stop=(kc == n - 1))

            # ---- lost_T (bf16) ----
            for Mb in range(4):
                for Nb in range(2):
                    ps = pp.tile([128, 512], f32, space="PSUM", tag="psA")
                    mm_acc(ps[:],
                           [Eb[:, kc, Mb * 128:(Mb + 1) * 128]
                            for kc in range(8)],
                           [ALTb[:, kc, Nb * 512:(Nb + 1) * 512]
                            for kc in range(8)])
                    nc.scalar.activation(
                        lostT_b[:, Mb, Nb * 512:(Nb + 1) * 512], ps[:],
                        AF.Identity)

            # ---- L1 rows (bf16) -> TU chunks of T1U ----
            for Mb in range(8):
                psA = pp.tile([128, 512], f32, space="PSUM", tag="psA")
                psB = pp.tile([128, 512], f32, space="PSUM", tag="psB")
                lh = [ETb[:, kc, Mb * 128:(Mb + 1) * 128] for kc in range(4)]
                mm_acc(psA[:], lh, [lostT_b[:, kc, 0:512] for kc in range(4)])
                mm_acc(psB[:], lh,
                       [lostT_b[:, kc, 512:1024] for kc in range(4)])
                m2 = ptiny.tile([128, 2], f32, tag="m2")
                nc.vector.tensor_reduce(m2[:, 0:1], psA[:], axis=X, op=OP.max)
                nc.vector.tensor_reduce(m2[:, 1:2], psB[:], axis=X, op=OP.max)
                mx = ptiny.tile([128, 1], f32, tag="mx")
                nc.vector.tensor_tensor(mx[:], m2[:, 0:1], m2[:, 1:2],
                                        op=OP.max)
                nmx = ptiny.tile([128, 1], f32, tag="nmx")
                nc.vector.tensor_scalar_mul(nmx[:], mx[:], -1.0)
                s2 = ptiny.tile([128, 2], f32, tag="s2")
                eo = pstg.tile([128, 512], f16, tag="peo")
                nc.scalar.activation(eo[:], psA[:], AF.Exp,
                                     bias=nmx[:, 0:1], accum_out=s2[:, 0:1])
                eo = pstg.tile([128, 512], f16, tag="peo")
                nc.scalar.activation(eo[:], psB[:], AF.Exp,
                                     bias=nmx[:, 0:1], accum_out=s2[:, 1:2])
                s = ptiny.tile([128, 1], f32, tag="s")
                nc.vector.tensor_tensor(s[:], s2[:, 0:1], s2[:, 1:2],
                                        op=OP.add)
                ls = ptiny.tile([128, 1], f32, tag="ls")
                nc.scalar.activation(ls[:], s[:], AF.Ln)
                bU = ptiny.tile([128, 1], f32, tag="bU")
                nc.vector.tensor_tensor(bU[:], mx[:], ls[:], op=OP.add)
                bU8 = ptiny.tile([128, 1], f32, tag="bU8")
                nc.vector.tensor_scalar_mul(bU8[:], bU[:], 0.8)
                tu = pstg.tile([128, LO], f16, tag="tu")
                nc.scalar.activation(tu[:, 0:512], psA[:], AF.Identity,
                                     bias=bU8[:, 0:1], scale=-0.8)
                nc.scalar.activation(tu[:, 512:1024], psB[:], AF.Identity,
                                     bias=bU8[:, 0:1], scale=-0.8)
                r0, r1 = Mb * 128, (Mb + 1) * 128
                nc.sync.dma_start(T1U[r0:r1, 2 * LO:3 * LO], tu[:])
                nc.sync.dma_start(T1U[K + r0:K + r1, 2 * LO:3 * LO], tu[:])
                nc.sync.dma_start(T1U[K + r0:K + r1, 0:2 * LO], zero2k[:])

            nc.sync.dma_start(T0P[K:K + 1, :], zero2k[0:1, :])
            nc.sync.dma_start(T2P[K:K + 1, :], zero2k[0:1, :])

            # ---- M_kw^b and P tables (k order matches gather order) ----
            for k in (1, 0, 2):
                Ms = {}
                for b in ("s", "i"):
                    Wt = pstg.tile([128, 4, D], bf16, tag="Wt")
                    nc.sync.dma_start(
                        Wt[:],
                        W_in[(b, k)][:].rearrange("(kc p) d -> p kc d", p=128))
                    M = pstg.tile([128, 4, LO], bf16, tag=f"M{b}")
                    for Mb in range(4):
                        for Nb in range(2):
                            ps = pp.tile([128, 512], f32, space="PSUM",
                                         tag="psA")
                            mm_acc(ps[:],
                                   [Wt[:, kc, Mb * 128:(Mb + 1) * 128]
                                    for kc in range(4)],
                                   [lostT_b[:, kc, Nb * 512:(Nb + 1) * 512]
                                    for kc in range(4)])
                            nc.scalar.activation(
                                M[:, Mb, Nb * 512:(Nb + 1) * 512], ps[:],
                                AF.Identity)
                    Ms[b] = M
                for Mb in range(8):
                    stage = pstg.tile([128, 2 * LO], f16, tag="stage")
                    lh = [ETb[:, kc, Mb * 128:(Mb + 1) * 128]
                          for kc in range(4)]
                    for bi, b in enumerate(("s", "i")):
                        psA = pp.tile([128, 512], f32, space="PSUM", tag="psA")
                        psB = pp.tile([128, 512], f32, space="PSUM", tag="psB")
                        mm_acc(psA[:], lh,
                               [Ms[b][:, kc, 0:512] for kc in range(4)])
                        mm_acc(psB[:], lh,
                               [Ms[b][:, kc, 512:1024] for kc in range(4)])
                        m2 = ptiny.tile([128, 2], f32, tag="m2")
                        nc.vector.tensor_reduce(m2[:, 0:1], psA[:], axis=X,
                                                op=OP.max)
                        nc.vector.tensor_reduce(m2[:, 1:2], psB[:], axis=X,
                                                op=OP.max)
                        mx = ptiny.tile([128, 1], f32, tag="mx")
                        nc.vector.tensor_tensor(mx[:], m2[:, 0:1], m2[:, 1:2],
                                                op=OP.max)
                        b02 = ptiny.tile([128, 1], f32, tag="b02")
                        nc.vector.tensor_scalar_mul(b02[:], mx[:], CW)
                        co = bi * LO
                        nc.scalar.activation(stage[:, co:co + 512], psA[:],
                                             AF.Identity, bias=b02[:, 0:1],
                                             scale=-CW)
                        nc.scalar.activation(stage[:, co + 512:co + 1024],
                                             psB[:], AF.Identity,
                                             bias=b02[:, 0:1], scale=-CW)
                    r0, r1 = Mb * 128, (Mb + 1) * 128
                    if k == 0:
                        nc.sync.dma_start(T0P[r0:r1, :], stage[:])
                    elif k == 2:
                        nc.sync.dma_start(T2P[r0:r1, :], stage[:])
                    else:
                        nc.sync.dma_start(T1U[r0:r1, 0:2 * LO], stage[:])

            # ---- exact f32: lost, per-core L1 shard -> alignment ----
            E32 = pre.tile([128, 8, D], f32)
            nc.sync.dma_start(
                E32[:], E_in[:].rearrange("(kc p) d -> p kc d", p=128))
            ALT32 = pre.tile([128, 8, LO], f32)
            nc.sync.dma_start(
                ALT32[:], ALT_in[:].rearrange("(kc p) d -> p kc d", p=128))
            ETsh = pre.tile([128, 4, 128], f32)
            nc.sync.dma_start(
                ETsh[:], ETsh_in[:].rearrange("(kc p) d -> p kc d", p=128))
            lostT32 = pre.tile([128, 4, LO], f32)
            for Mb in range(4):
                for Nb in range(2):
                    ps = pp.tile([128, 512], f32, space="PSUM", tag="psA")
                    mm_acc(ps[:],
                           [E32[:, kc, Mb * 128:(Mb + 1) * 128]
                            for kc in range(8)],
                           [ALT32[:, kc, Nb * 512:(Nb + 1) * 512]
                            for kc in range(8)])
                    nc.scalar.activation(
                        lostT32[:, Mb, Nb * 512:(Nb + 1) * 512], ps[:],
                        AF.Identity)
            psA = pp.tile([128, 512], f32, space="PSUM", tag="psA")
            psB = pp.tile([128, 512], f32, space="PSUM", tag="psB")
            lh = [ETsh[:, kc, :] for kc in range(4)]
            mm_acc(psA[:], lh, [lostT32[:, kc, 0:512] for kc in range(4)])
            mm_acc(psB[:], lh, [lostT32[:, kc, 512:1024] for kc in range(4)])
            m2 = ptiny.tile([128, 2], f32, tag="m2")
            nc.vector.tensor_reduce(m2[:, 0:1], psA[:], axis=X, op=OP.max)
            nc.vector.tensor_reduce(m2[:, 1:2], psB[:], axis=X, op=OP.max)
            mx = ptiny.tile([128, 1], f32, tag="mx")
            nc.vector.tensor_tensor(mx[:], m2[:, 0:1], m2[:, 1:2], op=OP.max)
            nmx = ptiny.tile([128, 1], f32, tag="nmx")
            nc.vector.tensor_scalar_mul(nmx[:], mx[:], -1.0)
            s2 = ptiny.tile([128, 2], f32, tag="s2")
            ex = pre.tile([128, 1024], f32)
            nc.scalar.activation(ex[:, 0:512], psA[:], AF.Exp,
                                 bias=nmx[:, 0:1], accum_out=s2[:, 0:1])
            nc.scalar.activation(ex[:, 512:1024], psB[:], AF.Exp,
                                 bias=nmx[:, 0:1], accum_out=s2[:, 1:2])
            s = ptiny.tile([128, 1], f32, tag="s")
            nc.vector.tensor_tensor(s[:], s2[:, 0:1], s2[:, 1:2], op=OP.add)
            rs = ptiny.tile([128, 1], f32, tag="rs")
            nc.vector.reciprocal(rs[:], s[:])
            alw = pre.tile([128, 1024], f32)
            nc.vector.tensor_scalar(alw[:], ex[:], scalar1=rs[:, 0:1],
                                    scalar2=None, op0=OP.mult)
            nc.sync.dma_start(ali_out[:], alw[:])

        # ===================== MAIN LOOP =====================
        with tc.tile_pool(name="gath", bufs=2) as gp, \
             tc.tile_pool(name="outp", bufs=2) as op_, \
             tc.tile_pool(name="msc", bufs=2) as msc, \
             tc.tile_pool(name="mtiny", bufs=4) as mtiny:
            for st in range(NST):
                sl = slice(st * MW, (st + 1) * MW)
                G1 = gp.tile([128, CT, 3 * LO], f16, tag="G1")
                nc.gpsimd.dma_gather(out_ap=G1[:], in_ap=T1U[:],
                                     idxs_ap=idx_sb["idx1"][:, sl],
                                     num_idxs=NI, num_idxs_reg=NI,
                                     elem_size=3 * LO)
                G0 = gp.tile([128, CT, 2 * LO], f16, tag="G0")
                nc.gpsimd.dma_gather(out_ap=G0[:], in_ap=T0P[:],
                                     idxs_ap=idx_sb["idx0"][:, sl],
                                     num_idxs=NI, num_idxs_reg=NI,
                                     elem_size=2 * LO)
                G2 = gp.tile([128, CT, 2 * LO], f16, tag="G2")
                nc.gpsimd.dma_gather(out_ap=G2[:], in_ap=T2P[:],
                                     idxs_ap=idx_sb["idx2"][:, sl],
                                     num_idxs=NI, num_idxs_reg=NI,
                                     elem_size=2 * LO)
                nc.vector.tensor_tensor(G0[:], G0[:], G2[:], op=OP.add)
                nc.vector.tensor_tensor(G0[:], G0[:], G1[:, :, 0:2 * LO],
                                        op=OP.add)
                # stride-4 sampled min of A_G (exact-enough exp guard)
                mn8 = mtiny.tile([128, 2 * CT], f32, tag="mn8")
                for c in range(CT):
                    for bi in range(2):
                        j = 2 * c + bi
                        nc.vector.tensor_reduce(
                            mn8[:, j:j + 1],
                            G0[:, c, bi * LO:(bi + 1) * LO:4],
                            axis=X, op=OP.min)
                # bias = mn/CW - 64*ln2: prescales the exp sums by 2^-64 so
                # sampled-min overshoot tails stay inside Ln's 2^64 domain
                bn8 = mtiny.tile([128, 2 * CT], f32, tag="bn8")
                nc.vector.tensor_scalar(bn8[:], mn8[:], scalar1=1.0 / CW,
                                        scalar2=-64.0 * float(np.log(2.0)),
                                        op0=OP.mult, op1=OP.add)
                s8 = mtiny.tile([128, 2 * CT], f32, tag="s8")
                for c in range(CT):
                    for bi in range(2):
                        j = 2 * c + bi
                        eo = msc.tile([128, LO], f32, tag="eo")
                        nc.scalar.activation(
                            eo[:], G0[:, c, bi * LO:(bi + 1) * LO], AF.Exp,
                            scale=-1.0 / CW, bias=bn8[:, j:j + 1],
                            accum_out=s8[:, j:j + 1])
                cls = mtiny.tile([128, 2 * CT], f32, tag="cls")
                nc.scalar.activation(cls[:], s8[:], AF.Ln)
                cls2 = mtiny.tile([128, 2 * CT], f32, tag="cls2")
                nc.vector.tensor_scalar(cls2[:], cls[:], scalar1=CW,
                                        scalar2=CW * 64.0 * float(np.log(2.0)),
                                        op0=OP.mult, op1=OP.add)
                nc.vector.tensor_tensor(cls2[:], cls2[:], mn8[:],
                                        op=OP.subtract)
                # O[p, c, b, :] = (A_G + cls2) + (-0.8*U) broadcast over b
                O = op_.tile([128, CT, 2, LO], f16, tag="O")
                for c in range(CT):
                    for bi in range(2):
                        j = 2 * c + bi
                        nc.vector.tensor_scalar(
                            O[:, c, bi, :], G0[:, c, bi * LO:(bi + 1) * LO],
                            scalar1=cls2[:, j:j + 1], scalar2=None,
                            op0=OP.add)
                u_sl = G1[:, :, 2 * LO:3 * LO]
                u_bc = bass.AP(u_sl.tensor, u_sl.offset,
                               [u_sl.ap[0], u_sl.ap[1], [0, 2], u_sl.ap[2]])
                nc.vector.tensor_tensor(O[:], O[:], u_bc, op=OP.add)
                sv = sub_out[st * NI:(st + 1) * NI, :].rearrange(
                    "(c p) n -> p c n", p=128)
                iv = ins_out[st * NI:(st + 1) * NI, :].rearrange(
                    "(c p) n -> p c n", p=128)
                nc.sync.dma_start(sv, O[:, :, 0, :])
                nc.sync.dma_start(iv, O[:, :, 1, :])

    nc.compile()
    return nc


def _get_nc():
    if "nc" not in _CACHE:
        _CACHE["nc"] = _build_nc()
    return _CACHE["nc"]


def _wrap_idx(a):
    """Flat [PROWS] int array -> [128, MW*NST] int16 gather-index layout."""
    out = np.empty((128, MW * NST), np.int16)
    for st in range(NST):
        w = a[st * NI:(st + 1) * NI].reshape(MW, 16).T.astype(np.int16)
        out[:, st * MW:(st + 1) * MW] = np.tile(w, (8, 1))
    return out


def host_prep(known_unit_emb, unit_aligner_weight, conv_w, conv_b,
              ins_conv_w, ins_conv_b, indexed_segments, vocab_length):
    """Build the per-core input maps (numpy only)."""
    E = np.asarray(known_unit_emb, np.float32)
    AL = np.asarray(unit_aligner_weight, np.float32)
    Wc = np.asarray(conv_w, np.float32)
    Wi = np.asarray(ins_conv_w, np.float32)
    idx = np.asarray(indexed_segments).astype(np.int64)
    ln = np.asarray(vocab_length).astype(np.int64)

    bf = ml_dtypes.bfloat16
    ET = np.ascontiguousarray(E.T)
    ALT = np.ascontiguousarray(AL.T)
    base = {
        "Eb": np.ascontiguousarray(E, bf),
        "ETb": np.ascontiguousarray(ET, bf),
        "ALTb": np.ascontiguousarray(ALT, bf),
        "E": E, "ALT": ALT,
    }
    for b, W in (("s", Wc), ("i", Wi)):
        for k in range(3):
            base[f"W{b}{k}"] = np.ascontiguousarray(W[:, :, k], bf)

    lpos = np.arange(L)
    valid = lpos[None, :] < ln[:, None]                      # [V, L]
    prev = np.full((V, L), K, np.int64)
    prev[:, 1:] = np.where(valid[:, :-1], idx[:, :-1], K)
    nxt = np.full((V, L), K, np.int64)
    nxt[:, :-1] = np.where(valid[:, 1:], idx[:, 1:], K)
    cur = np.where(valid, idx, K + idx)                      # T1U double-height

    in_maps = []
    for c in range(NCORES):
        vs = slice(c * VPC, (c + 1) * VPC)
        m = dict(base)
        m["ETsh"] = np.ascontiguousarray(ET[:, c * 128:(c + 1) * 128])
        for nm, arr, pad in (("idx0", prev, K), ("idx1", cur, K),
                             ("idx2", nxt, K)):
            flat = np.full(PROWS, pad, np.int64)
            flat[:ROWS] = arr[vs].reshape(-1)
            m[nm] = _wrap_idx(flat)
        in_maps.append(m)
    return in_maps


def assemble(results):
    sub = np.concatenate(
        [r["sub"][:ROWS].astype(np.float32).reshape(VPC, L, LO)
         for r in results], axis=0)
    ins = np.concatenate(
        [r["ins"][:ROWS].astype(np.float32).reshape(VPC, L, LO)
         for r in results], axis=0)
    ali = np.concatenate([r["ali"].astype(np.float32) for r in results],
                         axis=0)
    return sub, ins, ali


def kernel(**inputs):
    from concourse.bass_utils import run_bass_kernel_spmd
    nc = _get_nc()
    in_maps = host_prep(**inputs)
    res = run_bass_kernel_spmd(nc, in_maps, core_ids=list(range(NCORES)))
    return assemble(res.results)
